# revision 5
# baseline (speedup 1.0000x reference)
"""EXL3 trellis-quantized linear layer on 8 Trainium2 NeuronCores.

y = Had(Had(x*suh) @ dequant(trellis)) * svh + bias

Sharding: column-parallel over output features (N). Each of the 8 cores
dequants and multiplies its 1792-column shard; host concatenates.

Decode pipeline per weight (t = column-within-tile class, fixed shift r):
    pair  = (A << 16) | B                       A,B = trellis word pair
    state = (pair >> (16-r)) & 0xFFFF           DVE (i32 bitwise shr+and)
    st2   = state + DELTA16                     ACT (exact, < 2^17)
    g1    = st2 * Q   mod 2^32                  Pool TT (int32 exact)
    z     = g1 & 0x8FFF8FFF                     DVE
    z.hi  = (g1.hi + RHO16) & 0x8FFF            ACT add + DVE odd-mask
where DELTA16*Q == D (mod 2^16) and RHO16 corrects the high half, so
z == state*Q + D (mod 2^32) masked, exactly.  The masked z tile is bitcast
to fp16 and streamed to the PE as two rhs streams (lo/hi interleaved)
accumulating into the same PSUM bank.

Weight (j,t) of tile (Tk,Tn) sits at W[16Tk+j, 16Tn+t], so an output
column's weights share one t class. PSUM columns are produced t-major and
the output Hadamard uses a row-permuted H to compensate.
"""

import sys

if "/opt/trn_rl_repo" not in sys.path:
    sys.path.insert(0, "/opt/trn_rl_repo")

import numpy as np

import concourse.bacc as bacc
import concourse.mybir as mybir
from concourse import tile
from concourse.bass_utils import run_bass_kernel_spmd

AL = mybir.AluOpType
DT = mybir.dt

# problem geometry (hardcoded per contest contract)
K = 4096
N = 14336
BATCH = 8
NCORES = 8
TNC = (N // 16) // NCORES  # 112 trellis tile-cols per core
NC_COLS = TNC * 16  # 1792 out features per core
SLABS = [(0, 32), (32, 32), (64, 32), (96, 16)]  # (Tn offset, width)
KC = 32  # 128-row k-chunks

LCG_Q = 89226354
LCG_D = 64248484
DELTA16 = 14306  # DELTA16*Q ≡ D (mod 2^16)
RHO16 = 53288  # ((D - DELTA16*Q) >> 16) mod 2^16
MASK32 = np.int32(np.uint32(0x8FFF8FFF).astype(np.int64) - (1 << 32))
# how many of the 16 classes do the +DELTA16 on DVE instead of ACT
DELTA_ON_DVE = 0
# how many of the 16 classes do the +RHO16 on DVE instead of ACT
RHO_ON_DVE = 8

# per-class constants
CLS = []
for t in range(16):
    c = (3 * t) // 16
    r = 3 * t - 16 * c
    CLS.append((c, r))


def _hadamard128():
    h = np.array([[1.0]], dtype=np.float64)
    while h.shape[0] < 128:
        h = np.block([[h, h], [h, -h]])
    return (h / np.sqrt(128.0)).astype(np.float32)


def _perm_h():
    # psum col f' = t*8 + sub  <->  true in-block col sub*16 + t
    h = _hadamard128()
    pi = np.zeros(128, dtype=np.int64)
    for t in range(16):
        for sub in range(8):
            pi[t * 8 + sub] = sub * 16 + t
    return np.ascontiguousarray(h[pi, :])


_NC_CACHE = {}


def _build_program(variant=""):
    if variant in _NC_CACHE:
        return _NC_CACHE[variant]

    nc = bacc.Bacc("TRN2", target_bir_lowering=False, debug=False)

    d_pairs = nc.dram_tensor("pairs", [128, 3 * KC * TNC], DT.int32, kind="ExternalInput")
    d_xT = nc.dram_tensor("xT", [128, KC * BATCH], DT.float16, kind="ExternalInput")
    d_suhT = nc.dram_tensor("suhT", [128, KC], DT.float16, kind="ExternalInput")
    d_H = nc.dram_tensor("Hmat", [128, 128], DT.float32, kind="ExternalInput")
    d_HP = nc.dram_tensor("HP", [128, 128], DT.float32, kind="ExternalInput")
    d_ident = nc.dram_tensor("ident8", [8, 8], DT.float32, kind="ExternalInput")
    d_svh = nc.dram_tensor("svhb", [8, NC_COLS], DT.float16, kind="ExternalInput")
    d_bias = nc.dram_tensor("biasb", [8, NC_COLS], DT.float16, kind="ExternalInput")
    d_out = nc.dram_tensor("out", [8, NC_COLS], DT.float16, kind="ExternalOutput")

    with tile.TileContext(nc) as tc:
        with (
            tc.tile_pool(name="const", bufs=1) as cpool,
            tc.tile_pool(name="pairs", bufs=1) as ppool,
            tc.tile_pool(name="cls", bufs=2) as clspool,
            tc.tile_pool(name="lcg", bufs=2) as lcgpool,
            tc.tile_pool(name="zslab", bufs=2) as zpool,
            tc.tile_pool(name="zslab1", bufs=1) as zpool1,
            tc.tile_pool(name="outp", bufs=1) as opool,
            tc.tile_pool(name="psum", bufs=2, space="PSUM") as pspool,
            tc.tile_pool(name="psum_s", bufs=2, space="PSUM") as pspool_s,
        ):
            # ---- constants / small inputs ----
            pairs = ppool.tile([128, 3 * KC * TNC], DT.int32, tag="pairs")
            for cpl in range(3):
                w3 = KC * TNC
                sl = slice(cpl * w3, (cpl + 1) * w3)
                nc.sync.dma_start(pairs[:, sl], d_pairs[:, sl])
            t_xT = cpool.tile([128, KC * BATCH], DT.float16, tag="xT")
            t_suhT = cpool.tile([128, KC], DT.float16, tag="suhT")
            t_H = cpool.tile([128, 128], DT.float32, tag="H")
            t_HP = cpool.tile([128, 128], DT.float32, tag="HP")
            t_id8 = cpool.tile([8, 8], DT.float32, tag="id8")
            t_svh = cpool.tile([8, NC_COLS], DT.float16, tag="svh")
            t_bias = cpool.tile([8, NC_COLS], DT.float16, tag="bias")
            nc.sync.dma_start(t_xT[:], d_xT[:])
            nc.sync.dma_start(t_suhT[:], d_suhT[:])
            nc.sync.dma_start(t_H[:], d_H[:])
            nc.sync.dma_start(t_HP[:], d_HP[:])
            nc.sync.dma_start(t_id8[:], d_ident[:])
            nc.sync.dma_start(t_svh[:], d_svh[:])
            nc.sync.dma_start(t_bias[:], d_bias[:])

            t_q = cpool.tile([128, 1], DT.int32, tag="cq")
            nc.vector.memset(t_q[:], LCG_Q)
            t_delta = cpool.tile([128, 1], DT.float32, tag="cdelta")
            nc.vector.memset(t_delta[:], float(DELTA16))
            t_rho = cpool.tile([128, 1], DT.float32, tag="crho")
            nc.vector.memset(t_rho[:], float(RHO16))

            # ---- input rotation: xhT[j, kc*8+b] ----
            t_xsT = cpool.tile([128, KC * BATCH], DT.float32, tag="xsT")
            nc.vector.tensor_tensor(
                t_xsT[:].rearrange("p (kc b) -> p kc b", kc=KC),
                t_xT[:].rearrange("p (kc b) -> p kc b", kc=KC),
                t_suhT[:].unsqueeze(2).broadcast_to([128, KC, BATCH]),
                AL.mult,
            )
            ps_xh = pspool.tile([128, KC * BATCH], DT.float32, tag="ps_xh")
            nc.tensor.matmul(ps_xh[:], t_H[:], t_xsT[:], start=True, stop=True)
            t_xhT = cpool.tile([128, KC * BATCH], DT.float16, tag="xhT")
            nc.scalar.copy(t_xhT[:], ps_xh[:])

            t_out = opool.tile([8, NC_COLS], DT.float16, tag="outsb")
            t_yh = opool.tile([8, NC_COLS], DT.float32, tag="yhsb")

            # ---- main loop over Tn slabs ----
            for tn0, tnw in SLABS:
                fw = KC * tnw  # class-op free width
                tza = zpool.tile([128, 8 * KC * 32], DT.int32, tag="za")
                tzb = zpool1.tile([128, 8 * KC * 32], DT.int32, tag="zb")
                tzh = [tza, tzb]
                pview = pairs[:].rearrange("p (c kc tn) -> p c kc tn", c=3, kc=KC)
                for t16, (c, r) in enumerate(CLS):
                    pair_v = pview[:, c, :, tn0 : tn0 + tnw]
                    # state = (pair >> (16-r)) & 0xFFFF   (bitwise, exact)
                    t_st = clspool.tile([128, fw], DT.int32, tag="st")
                    nc.vector.tensor_scalar(
                        t_st[:], pair_v, 16 - r, 0xFFFF,
                        AL.logical_shift_right, AL.bitwise_and,
                    )
                    # st2 = state + DELTA16  (exact: < 2^17)
                    t_st2 = clspool.tile([128, fw], DT.int32, tag="st2")
                    if t16 < DELTA_ON_DVE:
                        nc.vector.tensor_scalar(
                            t_st2[:], t_st[:], float(DELTA16), None, AL.add
                        )
                    else:
                        nc.scalar.activation(
                            t_st2[:], t_st[:],
                            mybir.ActivationFunctionType.Identity,
                            bias=t_delta[:], scale=1.0,
                        )
                    # g1 = st2 * Q  (int32 exact wraparound on gpsimd)
                    t_g1 = lcgpool.tile([128, fw], DT.int32, tag="g1")
                    nc.gpsimd.tensor_tensor(
                        t_g1[:], t_st2[:], t_q[:].broadcast_to([128, fw]), AL.mult
                    )
                    # masked z into the interleaved stream tile
                    tzv = tzh[t16 // 8][:, (t16 % 8) * fw : (t16 % 8 + 1) * fw]
                    nc.vector.tensor_scalar(
                        tzv, t_g1[:], int(MASK32), None, AL.bitwise_and
                    )
                    # hi halves need +rho (mod 2^16) before masking:
                    # ACT adds rho on the odd int16 view, DVE masks and
                    # writes the odd halves back.
                    t_h32 = lcgpool.tile([128, fw], DT.int32, tag="h32")
                    zq_odd = t_g1[:].bitcast(DT.int16).rearrange(
                        "p (n x) -> p x n", x=2
                    )[:, 1]
                    if t16 < RHO_ON_DVE:
                        nc.vector.tensor_scalar(
                            t_h32[:], zq_odd, float(RHO16), None, AL.add
                        )
                    else:
                        nc.scalar.activation(
                            t_h32[:], zq_odd,
                            mybir.ActivationFunctionType.Identity,
                            bias=t_rho[:], scale=1.0,
                        )
                    tz_odd = tzv.bitcast(DT.int16).rearrange(
                        "p (n x) -> p x n", x=2
                    )[:, 1]
                    h32_lo = t_h32[:].bitcast(DT.int16).rearrange(
                        "p (n x) -> p x n", x=2
                    )[:, 0]
                    nc.vector.tensor_scalar(
                        tz_odd, h32_lo, 0x8FFF, None, AL.bitwise_and
                    )

                # ---- matmuls: 2 fp16 streams x 32 k-chunks ----
                nb = tnw // 8  # 128-col blocks in this slab
                ps_y = pspool.tile([8, 512], DT.float32, tag="ps_y")
                ps_yv = ps_y[:, : tnw * 16]
                pv = ps_yv.rearrange("p (b t sub) -> p b t sub", b=nb, t=16, sub=8)
                for half in range(2):
                    zf = tzh[half][:, : 8 * fw].bitcast(DT.float16).rearrange(
                        "p (t kc b sub x) -> p kc x b t sub",
                        t=8, kc=KC, b=nb, sub=8, x=2,
                    )
                    outv = pv[:, :, half * 8 : (half + 1) * 8, :]
                    n_mm = 2 * KC
                    i_mm = 0
                    for xi in range(2):
                        for kc in range(KC):
                            nc.tensor.matmul(
                                outv,
                                t_xhT[:, kc * BATCH : (kc + 1) * BATCH],
                                zf[:, kc, xi],
                                start=(i_mm == 0),
                                stop=(i_mm == n_mm - 1),
                                skip_group_check=True,
                            )
                            i_mm += 1

                # ---- tail: transpose + permuted Hadamard into t_yh ----
                t_y = clspool.tile([8, 512], DT.float32, tag="ysb")
                nc.scalar.copy(t_y[:, : tnw * 16], ps_yv)
                for bb in range(nb):
                    nblk = (tn0 // 8) + bb
                    ps_t = pspool_s.tile([128, 8], DT.float32, tag="ps_t")
                    nc.tensor.transpose(
                        ps_t[:], t_y[:, bb * 128 : (bb + 1) * 128], t_id8[:]
                    )
                    t_yT = clspool.tile([128, 8], DT.float32, tag="yT")
                    nc.scalar.copy(t_yT[:], ps_t[:])
                    ps_h = pspool_s.tile([8, 128], DT.float32, tag="ps_h")
                    nc.tensor.matmul(ps_h[:], t_yT[:], t_HP[:], start=True, stop=True)
                    nc.scalar.copy(
                        t_yh[:, nblk * 128 : (nblk + 1) * 128], ps_h[:]
                    )

            # ---- batched svh mult + bias add over the full shard ----
            t_ysv = opool.tile([8, NC_COLS], DT.float32, tag="ysv")
            nc.vector.tensor_tensor(t_ysv[:], t_yh[:], t_svh[:], AL.mult)
            nc.vector.tensor_tensor(t_out[:], t_ysv[:], t_bias[:], AL.add)
            nc.sync.dma_start(d_out[:], t_out[:])

    nc.compile()
    _NC_CACHE[variant] = nc
    return nc


def _prep_core_inputs(x, trellis, suh, svh, bias, core):
    tshard = trellis[:, core * TNC : (core + 1) * TNC, :]  # [256, 112, 48]
    j = np.arange(16)
    pairs = np.empty((128, 3 * KC * TNC), dtype=np.int32)
    for c in range(3):
        wA = (3 * j + c) % 48
        wB = (3 * j + c + 1) % 48
        plA = tshard[:, :, wA].astype(np.uint32)  # [256 Tk, 112 Tn, 16 j]
        plB = tshard[:, :, wB].astype(np.uint32)
        pl = (plA << 16) | plB
        # -> [p=16*tk8+j, kc, Tn]
        arr = pl.reshape(KC, 8, TNC, 16)  # [kc, tk8, Tn, j]
        arr = arr.transpose(1, 3, 0, 2).reshape(128, KC * TNC)
        pairs[:, c * KC * TNC : (c + 1) * KC * TNC] = arr.view(np.int32)

    # xT[p, kc*8+b] = x[b, kc*128+p]
    xT = np.ascontiguousarray(
        x.reshape(BATCH, KC, 128).transpose(2, 1, 0).reshape(128, KC * BATCH)
    )
    suhT = np.ascontiguousarray(suh.reshape(KC, 128).T)  # [128, 32]

    svh_s = svh[core * NC_COLS : (core + 1) * NC_COLS].astype(np.float32)
    bias_s = bias[core * NC_COLS : (core + 1) * NC_COLS].astype(np.float32)

    return {
        "pairs": pairs,
        "xT": xT,
        "suhT": suhT,
        "Hmat": _hadamard128(),
        "HP": _perm_h(),
        "ident8": np.eye(8, dtype=np.float32),
        "svhb": np.ascontiguousarray(np.broadcast_to(svh_s.astype(np.float16), (8, NC_COLS))),
        "biasb": np.ascontiguousarray(np.broadcast_to(bias_s.astype(np.float16), (8, NC_COLS))),
    }


def kernel(x, trellis, suh, svh, bias):
    x = np.asarray(x)
    trellis = np.asarray(trellis).astype(np.uint16)
    suh = np.asarray(suh)
    svh = np.asarray(svh)
    bias = np.asarray(bias)

    nc = _build_program()
    in_maps = [
        _prep_core_inputs(x, trellis, suh, svh, bias, core) for core in range(NCORES)
    ]
    res = run_bass_kernel_spmd(nc, in_maps, core_ids=list(range(NCORES)))
    global LAST_RUN
    LAST_RUN = res
    out = np.concatenate([res.results[c]["out"] for c in range(NCORES)], axis=1)
    return out.astype(np.float16)


LAST_RUN = None


if __name__ == "__main__":
    import reference as ref
    import jax.numpy as jnp

    inputs = {k: np.asarray(v) for k, v in ref.setup_inputs().items()}
    expected = np.asarray(ref.reference(**{k: jnp.asarray(v) for k, v in inputs.items()}))
    got = kernel(**inputs)
    e = np.linalg.norm(got.astype(np.float32) - expected.astype(np.float32))
    n = np.linalg.norm(expected.astype(np.float32))
    print("Relative error:", e / n)


# revision 8
# speedup vs baseline: 1.3186x; 1.3186x over previous
"""EXL3 trellis-quantized linear layer on 8 Trainium2 NeuronCores.

y = Had(Had(x*suh) @ dequant(trellis)) * svh + bias

Sharding: column-parallel over output features (N). Each of the 8 cores
dequants and multiplies its 1792-column shard; host concatenates.

Decode pipeline per weight (t = column-within-tile class, fixed shift r):
    pair  = (A << 16) | B                       A,B = trellis word pair
    state = (pair >> (16-r)) & 0xFFFF           DVE (i32 bitwise shr+and)
    st2   = state + DELTA16                     ACT (exact, < 2^17)
    g1    = st2 * Q   mod 2^32                  Pool TT (int32 exact)
    z     = g1 & 0x8FFF8FFF                     DVE
    z.hi  = (g1.hi + RHO16) & 0x8FFF            ACT add + DVE odd-mask
where DELTA16*Q == D (mod 2^16) and RHO16 corrects the high half, so
z == state*Q + D (mod 2^32) masked, exactly.  The masked z tile is bitcast
to fp16 and streamed to the PE as two rhs streams (lo/hi interleaved)
accumulating into the same PSUM bank.

The 64 (slab, class) items are software-pipelined: each engine's in-order
queue sees stage k of item i next to stage k-1 of item i+1, so no engine
stalls waiting for a cross-engine dependency of the same item.

Weight (j,t) of tile (Tk,Tn) sits at W[16Tk+j, 16Tn+t], so an output
column's weights share one t class. PSUM columns are produced t-major and
the output Hadamard uses a row-permuted H to compensate.
"""

import sys

if "/opt/trn_rl_repo" not in sys.path:
    sys.path.insert(0, "/opt/trn_rl_repo")

import numpy as np

import concourse.bacc as bacc
import concourse.mybir as mybir
from concourse import tile
from concourse.bass_utils import run_bass_kernel_spmd

AL = mybir.AluOpType
DT = mybir.dt

# problem geometry (hardcoded per contest contract)
K = 4096
N = 14336
BATCH = 8
NCORES = 8
TNC = (N // 16) // NCORES  # 112 trellis tile-cols per core
NC_COLS = TNC * 16  # 1792 out features per core
SLABS = [(0, 32), (32, 32), (64, 32), (96, 16)]  # (Tn offset, width)
KC = 32  # 128-row k-chunks

LCG_Q = 89226354
LCG_D = 64248484
DELTA16 = 14306  # DELTA16*Q ≡ D (mod 2^16)
RHO16 = 53288  # ((D - DELTA16*Q) >> 16) mod 2^16
MASK32 = np.int32(np.uint32(0x8FFF8FFF).astype(np.int64) - (1 << 32))

# stage lags for the software pipeline (item i emits stage S at i+LAG[S])
LAG_D = 1  # ACT delta
LAG_M = 2  # Pool mult
LAG_K = 3  # DVE mask
LAG_R = 3  # ACT rho
LAG_O = 4  # DVE odd write-back
MAXLAG = 4
TAIL_DELAY = 10  # items into the next slab before emitting a slab's tail

# per-class constants
CLS = []
for t in range(16):
    c = (3 * t) // 16
    r = 3 * t - 16 * c
    CLS.append((c, r))

# slab-major pairs layout offsets (in i32 elements per partition)
SLAB_OFF = []
_off = 0
for _tn0, _tnw in SLABS:
    SLAB_OFF.append(_off)
    _off += 3 * KC * _tnw
PAIRS_W = _off  # 3*KC*TNC


def _hadamard128():
    h = np.array([[1.0]], dtype=np.float64)
    while h.shape[0] < 128:
        h = np.block([[h, h], [h, -h]])
    return (h / np.sqrt(128.0)).astype(np.float32)


def _perm_h():
    # psum col f' = t*8 + sub  <->  true in-block col sub*16 + t
    h = _hadamard128()
    pi = np.zeros(128, dtype=np.int64)
    for t in range(16):
        for sub in range(8):
            pi[t * 8 + sub] = sub * 16 + t
    return np.ascontiguousarray(h[pi, :])


_NC_CACHE = {}


def _build_program(variant=""):
    if variant in _NC_CACHE:
        return _NC_CACHE[variant]

    nc = bacc.Bacc("TRN2", target_bir_lowering=False, debug=False)

    d_pairs = nc.dram_tensor("pairs", [128, PAIRS_W], DT.int32, kind="ExternalInput")
    d_xT = nc.dram_tensor("xT", [128, KC * BATCH], DT.float16, kind="ExternalInput")
    d_suhT = nc.dram_tensor("suhT", [128, KC], DT.float16, kind="ExternalInput")
    d_H = nc.dram_tensor("Hmat", [128, 128], DT.float32, kind="ExternalInput")
    d_HP = nc.dram_tensor("HP", [128, 128], DT.float32, kind="ExternalInput")
    d_ident = nc.dram_tensor("ident8", [8, 8], DT.float32, kind="ExternalInput")
    d_svh = nc.dram_tensor("svhb", [8, NC_COLS], DT.float16, kind="ExternalInput")
    d_bias = nc.dram_tensor("biasb", [8, NC_COLS], DT.float16, kind="ExternalInput")
    d_out = nc.dram_tensor("out", [8, NC_COLS], DT.float16, kind="ExternalOutput")

    with tile.TileContext(nc) as tc:
        with (
            tc.tile_pool(name="const", bufs=1) as cpool,
            tc.tile_pool(name="pairs", bufs=2) as ppool,
            tc.tile_pool(name="stp", bufs=4) as stpool,
            tc.tile_pool(name="st2p", bufs=3) as st2pool,
            tc.tile_pool(name="g1p", bufs=3) as g1pool,
            tc.tile_pool(name="h32p", bufs=3) as h32pool,
            tc.tile_pool(name="tail", bufs=2) as tailpool,
            tc.tile_pool(name="zslab", bufs=2) as zpool,
            tc.tile_pool(name="zslab1", bufs=1) as zpool1,
            tc.tile_pool(name="outp", bufs=1) as opool,
            tc.tile_pool(name="psum", bufs=2, space="PSUM") as pspool,
            tc.tile_pool(name="psum_s", bufs=2, space="PSUM") as pspool_s,
        ):
            # ---- constants / small inputs ----
            t_xT = cpool.tile([128, KC * BATCH], DT.float16, tag="xT")
            t_suhT = cpool.tile([128, KC], DT.float16, tag="suhT")
            t_H = cpool.tile([128, 128], DT.float32, tag="H")
            t_HP = cpool.tile([128, 128], DT.float32, tag="HP")
            t_id8 = cpool.tile([8, 8], DT.float32, tag="id8")
            t_svh = cpool.tile([8, NC_COLS], DT.float16, tag="svh")
            t_bias = cpool.tile([8, NC_COLS], DT.float16, tag="bias")
            nc.sync.dma_start(t_xT[:], d_xT[:])
            nc.sync.dma_start(t_suhT[:], d_suhT[:])
            nc.sync.dma_start(t_H[:], d_H[:])
            nc.sync.dma_start(t_HP[:], d_HP[:])
            nc.sync.dma_start(t_id8[:], d_ident[:])
            nc.sync.dma_start(t_svh[:], d_svh[:])
            nc.sync.dma_start(t_bias[:], d_bias[:])

            t_q = cpool.tile([128, 1], DT.int32, tag="cq")
            nc.vector.memset(t_q[:], LCG_Q)
            t_delta = cpool.tile([128, 1], DT.float32, tag="cdelta")
            nc.vector.memset(t_delta[:], float(DELTA16))
            t_rho = cpool.tile([128, 1], DT.float32, tag="crho")
            nc.vector.memset(t_rho[:], float(RHO16))

            # ---- input rotation: xhT[j, kc*8+b] ----
            t_xsT = cpool.tile([128, KC * BATCH], DT.float32, tag="xsT")
            nc.vector.tensor_tensor(
                t_xsT[:].rearrange("p (kc b) -> p kc b", kc=KC),
                t_xT[:].rearrange("p (kc b) -> p kc b", kc=KC),
                t_suhT[:].unsqueeze(2).broadcast_to([128, KC, BATCH]),
                AL.mult,
            )
            ps_xh = pspool.tile([128, KC * BATCH], DT.float32, tag="ps_xh")
            nc.tensor.matmul(ps_xh[:], t_H[:], t_xsT[:], start=True, stop=True)
            t_xhT = cpool.tile([128, KC * BATCH], DT.float16, tag="xhT")
            nc.scalar.copy(t_xhT[:], ps_xh[:])

            t_out = opool.tile([8, NC_COLS], DT.float16, tag="outsb")
            t_yh = opool.tile([8, NC_COLS], DT.float32, tag="yhsb")

            # ---- software-pipelined decode over flat (slab, class) items ----
            items = []
            for s in range(len(SLABS)):
                for t16 in range(16):
                    items.append((s, t16))
            NI = len(items)

            slab_state = {}  # s -> dict(pairs tile, z tiles, psum)
            st_of = {}  # item idx -> tiles

            def ensure_slab(s):
                if s in slab_state:
                    return slab_state[s]
                tn0, tnw = SLABS[s]
                w = 3 * KC * tnw
                tp = ppool.tile([128, 3 * KC * 32], DT.int32, tag="pairs", name="tpairs")
                wc = w // 3
                for cpl in range(3):
                    nc.sync.dma_start(
                        tp[:, cpl * wc : (cpl + 1) * wc],
                        d_pairs[:, SLAB_OFF[s] + cpl * wc : SLAB_OFF[s] + (cpl + 1) * wc],
                    )
                ss = {"pairs": tp, "w": w}
                slab_state[s] = ss
                return ss

            def emit_E(j):
                s, t16 = items[j]
                ss = ensure_slab(s)
                if t16 == 0 and s + 1 < len(SLABS):
                    ensure_slab(s + 1)  # prefetch next slab's pairs
                tn0, tnw = SLABS[s]
                fw = KC * tnw
                c, r = CLS[t16]
                pv = ss["pairs"][:, : ss["w"]].rearrange(
                    "p (c kc tn) -> p c kc tn", c=3, kc=KC
                )
                t_st = stpool.tile([128, KC * 32], DT.int32, tag="st")
                nc.vector.tensor_scalar(
                    t_st[:, :fw], pv[:, c], 16 - r, 0xFFFF,
                    AL.logical_shift_right, AL.bitwise_and,
                )
                st_of[j] = {"st": t_st, "fw": fw}

            def emit_D(j):
                io = st_of[j]
                fw = io["fw"]
                t_st2 = st2pool.tile([128, KC * 32], DT.int32, tag="st2")
                nc.scalar.activation(
                    t_st2[:, :fw], io["st"][:, :fw],
                    mybir.ActivationFunctionType.Identity,
                    bias=t_delta[:], scale=1.0,
                )
                io["st2"] = t_st2

            def emit_M(j):
                io = st_of[j]
                fw = io["fw"]
                t_g1 = g1pool.tile([128, KC * 32], DT.int32, tag="g1")
                nc.gpsimd.tensor_tensor(
                    t_g1[:, :fw], io["st2"][:, :fw],
                    t_q[:].broadcast_to([128, fw]), AL.mult,
                )
                io["g1"] = t_g1

            def z_view(j):
                s, t16 = items[j]
                io = st_of[j]
                fw = io["fw"]
                tz = slab_state[s]["tza"] if t16 < 8 else slab_state[s]["tzb"]
                return tz[:, (t16 % 8) * fw : (t16 % 8 + 1) * fw]

            def emit_K(j):
                s, t16 = items[j]
                io = st_of[j]
                fw = io["fw"]
                ss = slab_state[s]
                if "tza" not in ss:
                    ss["tza"] = zpool.tile([128, 8 * KC * 32], DT.int32, tag="za", name="tza")
                    ss["tzb"] = zpool1.tile([128, 8 * KC * 32], DT.int32, tag="zb", name="tzb")
                nc.vector.tensor_scalar(
                    z_view(j), io["g1"][:, :fw], int(MASK32), None, AL.bitwise_and
                )

            def emit_R(j):
                io = st_of[j]
                fw = io["fw"]
                t_h32 = h32pool.tile([128, KC * 32], DT.int32, tag="h32")
                zq_odd = io["g1"][:, :fw].bitcast(DT.int16).rearrange(
                    "p (n x) -> p x n", x=2
                )[:, 1]
                nc.scalar.activation(
                    t_h32[:, :fw], zq_odd,
                    mybir.ActivationFunctionType.Identity,
                    bias=t_rho[:], scale=1.0,
                )
                io["h32"] = t_h32

            def emit_O(j):
                s, t16 = items[j]
                io = st_of[j]
                fw = io["fw"]
                tz_odd = z_view(j).bitcast(DT.int16).rearrange(
                    "p (n x) -> p x n", x=2
                )[:, 1]
                h32_lo = io["h32"][:, :fw].bitcast(DT.int16).rearrange(
                    "p (n x) -> p x n", x=2
                )[:, 0]
                nc.vector.tensor_scalar(
                    tz_odd, h32_lo, 0x8FFF, None, AL.bitwise_and
                )

            def emit_mms(s, half):
                tn0, tnw = SLABS[s]
                fw = KC * tnw
                nb = tnw // 8
                ss = slab_state[s]
                if "ps_y" not in ss:
                    ss["ps_y"] = pspool.tile([8, 512], DT.float32, tag="ps_y", name="ps_y")
                ps_yv = ss["ps_y"][:, : tnw * 16]
                pv = ps_yv.rearrange("p (b t sub) -> p b t sub", b=nb, t=16, sub=8)
                tz = ss["tza"] if half == 0 else ss["tzb"]
                zf = tz[:, : 8 * fw].bitcast(DT.float16).rearrange(
                    "p (t kc b sub x) -> p kc x b t sub",
                    t=8, kc=KC, b=nb, sub=8, x=2,
                )
                outv = pv[:, :, half * 8 : (half + 1) * 8, :]
                n_mm = 2 * KC
                i_mm = 0
                for xi in range(2):
                    for kc in range(KC):
                        nc.tensor.matmul(
                            outv,
                            t_xhT[:, kc * BATCH : (kc + 1) * BATCH],
                            zf[:, kc, xi],
                            start=(i_mm == 0),
                            stop=(i_mm == n_mm - 1),
                            skip_group_check=True,
                        )
                        i_mm += 1

            def emit_tail(s):
                tn0, tnw = SLABS[s]
                nb = tnw // 8
                ss = slab_state[s]
                ps_yv = ss["ps_y"][:, : tnw * 16]
                t_y = tailpool.tile([8, 512], DT.float32, tag="ysb")
                nc.scalar.copy(t_y[:, : tnw * 16], ps_yv)
                for bb in range(nb):
                    nblk = (tn0 // 8) + bb
                    ps_t = pspool_s.tile([128, 8], DT.float32, tag="ps_t")
                    nc.tensor.transpose(
                        ps_t[:], t_y[:, bb * 128 : (bb + 1) * 128], t_id8[:]
                    )
                    t_yT = tailpool.tile([128, 8], DT.float32, tag="yT")
                    nc.vector.tensor_copy(t_yT[:], ps_t[:])
                    ps_h = pspool_s.tile([8, 128], DT.float32, tag="ps_h")
                    nc.tensor.matmul(ps_h[:], t_yT[:], t_HP[:], start=True, stop=True)
                    nc.vector.tensor_copy(
                        t_yh[:, nblk * 128 : (nblk + 1) * 128], ps_h[:]
                    )

            ensure_slab(0)
            for i in range(NI + MAXLAG):
                if i < NI:
                    emit_E(i)
                if 0 <= i - LAG_D < NI:
                    emit_D(i - LAG_D)
                if 0 <= i - LAG_M < NI:
                    emit_M(i - LAG_M)
                if 0 <= i - LAG_K < NI:
                    emit_K(i - LAG_K)
                if 0 <= i - LAG_R < NI:
                    emit_R(i - LAG_R)
                if 0 <= i - LAG_O < NI:
                    j = i - LAG_O
                    emit_O(j)
                    s, t16 = items[j]
                    if t16 == 7:
                        emit_mms(s, 0)
                    elif t16 == 15:
                        emit_mms(s, 1)
                        if s == len(SLABS) - 1:
                            emit_tail(s)
                    if t16 == TAIL_DELAY and s > 0:
                        emit_tail(s - 1)
                    del st_of[j]

            # ---- batched svh mult + bias add over the full shard ----
            nc.vector.tensor_tensor(t_yh[:], t_yh[:], t_svh[:], AL.mult)
            nc.vector.tensor_tensor(t_out[:], t_yh[:], t_bias[:], AL.add)
            nc.sync.dma_start(d_out[:], t_out[:])

    nc.compile()
    _NC_CACHE[variant] = nc
    return nc


def _prep_core_inputs(x, trellis, suh, svh, bias, core):
    tshard = trellis[:, core * TNC : (core + 1) * TNC, :]  # [256, 112, 48]
    j = np.arange(16)
    pairs = np.empty((128, PAIRS_W), dtype=np.int32)
    for s, (tn0, tnw) in enumerate(SLABS):
        for c in range(3):
            wA = (3 * j + c) % 48
            wB = (3 * j + c + 1) % 48
            plA = tshard[:, tn0 : tn0 + tnw, wA].astype(np.uint32)  # [256, tnw, 16]
            plB = tshard[:, tn0 : tn0 + tnw, wB].astype(np.uint32)
            pl = (plA << 16) | plB
            arr = pl.reshape(KC, 8, tnw, 16)  # [kc, tk8, tn, j]
            arr = arr.transpose(1, 3, 0, 2).reshape(128, KC * tnw)
            off = SLAB_OFF[s] + c * KC * tnw
            pairs[:, off : off + KC * tnw] = arr.view(np.int32)

    # xT[p, kc*8+b] = x[b, kc*128+p]
    xT = np.ascontiguousarray(
        x.reshape(BATCH, KC, 128).transpose(2, 1, 0).reshape(128, KC * BATCH)
    )
    suhT = np.ascontiguousarray(suh.reshape(KC, 128).T)  # [128, 32]

    svh_s = svh[core * NC_COLS : (core + 1) * NC_COLS].astype(np.float16)
    bias_s = bias[core * NC_COLS : (core + 1) * NC_COLS].astype(np.float16)

    return {
        "pairs": pairs,
        "xT": xT,
        "suhT": suhT,
        "Hmat": _hadamard128(),
        "HP": _perm_h(),
        "ident8": np.eye(8, dtype=np.float32),
        "svhb": np.ascontiguousarray(np.broadcast_to(svh_s, (8, NC_COLS))),
        "biasb": np.ascontiguousarray(np.broadcast_to(bias_s, (8, NC_COLS))),
    }


def kernel(x, trellis, suh, svh, bias):
    x = np.asarray(x)
    trellis = np.asarray(trellis).astype(np.uint16)
    suh = np.asarray(suh)
    svh = np.asarray(svh)
    bias = np.asarray(bias)

    nc = _build_program()
    in_maps = [
        _prep_core_inputs(x, trellis, suh, svh, bias, core) for core in range(NCORES)
    ]
    res = run_bass_kernel_spmd(nc, in_maps, core_ids=list(range(NCORES)))
    global LAST_RUN
    LAST_RUN = res
    out = np.concatenate([res.results[c]["out"] for c in range(NCORES)], axis=1)
    return out.astype(np.float16)


LAST_RUN = None


if __name__ == "__main__":
    import reference as ref
    import jax.numpy as jnp

    inputs = {k: np.asarray(v) for k, v in ref.setup_inputs().items()}
    expected = np.asarray(ref.reference(**{k: jnp.asarray(v) for k, v in inputs.items()}))
    got = kernel(**inputs)
    e = np.linalg.norm(got.astype(np.float32) - expected.astype(np.float32))
    n = np.linalg.norm(expected.astype(np.float32))
    print("Relative error:", e / n)


# revision 14
# speedup vs baseline: 1.4205x; 1.0773x over previous
"""EXL3 trellis-quantized linear layer on 8 Trainium2 NeuronCores.

y = Had(Had(x*suh) @ dequant(trellis)) * svh + bias

Sharding: column-parallel over output features (N). Each of the 8 cores
dequants and multiplies its 1792-column shard; host concatenates.

Decode pipeline per weight (t = column-within-tile class, fixed shift r):
    pair  = (A << 16) | B                       A,B = trellis word pair
    state = (pair >> (16-r)) & 0xFFFF           DVE (i32 bitwise shr+and)
    st2   = state + DELTA16                     ACT (exact, < 2^17)
    g1    = st2 * Q   mod 2^32                  Pool TT (int32 exact)
    z     = g1 & 0x8FFF8FFF                     DVE
    z.hi  = (g1.hi + RHO16) & 0x8FFF            ACT add + DVE odd-mask
where DELTA16*Q == D (mod 2^16) and RHO16 corrects the high half, so
z == state*Q + D (mod 2^32) masked, exactly.  The masked z tile is bitcast
to fp16 and streamed to the PE as two rhs streams (lo/hi interleaved)
accumulating into the same PSUM bank.

The 64 (slab, class) items are software-pipelined: each engine's in-order
queue sees stage k of item i next to stage k-1 of item i+1, so no engine
stalls waiting for a cross-engine dependency of the same item.

Weight (j,t) of tile (Tk,Tn) sits at W[16Tk+j, 16Tn+t], so an output
column's weights share one t class. PSUM columns are produced t-major and
the output Hadamard uses a row-permuted H to compensate.
"""

import sys

if "/opt/trn_rl_repo" not in sys.path:
    sys.path.insert(0, "/opt/trn_rl_repo")

import numpy as np

import concourse.bacc as bacc
import concourse.mybir as mybir
from concourse import tile
from concourse.bass_utils import run_bass_kernel_spmd

AL = mybir.AluOpType
DT = mybir.dt

# problem geometry (hardcoded per contest contract)
K = 4096
N = 14336
BATCH = 8
NCORES = 8
TNC = (N // 16) // NCORES  # 112 trellis tile-cols per core
NC_COLS = TNC * 16  # 1792 out features per core
SLABS = [(0, 16), (16, 32), (48, 32), (80, 32)]  # (Tn offset, width)
KC = 32  # 128-row k-chunks

LCG_Q = 89226354
LCG_D = 64248484
DELTA16 = 14306  # DELTA16*Q ≡ D (mod 2^16)
RHO16 = 53288  # ((D - DELTA16*Q) >> 16) mod 2^16
MASK32 = np.int32(np.uint32(0x8FFF8FFF).astype(np.int64) - (1 << 32))

# stage lags for the software pipeline (item i emits stage S at i+LAG[S])
LAG_D = 1  # ACT delta
LAG_M = 2  # Pool mult
LAG_R = 3  # ACT rho (reads unmasked z: must precede K)
LAG_K = 4  # DVE mask (in place on z)
LAG_O = 5  # DVE odd write-back
MAXLAG = 5
TAIL_DELAY = 5  # pair-slots into the next slab before emitting a slab's tail

# per-class constants
CLS = []
for t in range(16):
    c = (3 * t) // 16
    r = 3 * t - 16 * c
    CLS.append((c, r))

# slab-major pairs layout offsets (in i32 elements per partition)
SLAB_OFF = []
_off = 0
for _tn0, _tnw in SLABS:
    SLAB_OFF.append(_off)
    _off += 3 * KC * _tnw
PAIRS_W = _off  # 3*KC*TNC


def _hadamard128():
    h = np.array([[1.0]], dtype=np.float64)
    while h.shape[0] < 128:
        h = np.block([[h, h], [h, -h]])
    return (h / np.sqrt(128.0)).astype(np.float32)


def _perm_h():
    # psum col f' = t*8 + sub  <->  true in-block col sub*16 + t
    h = _hadamard128()
    pi = np.zeros(128, dtype=np.int64)
    for t in range(16):
        for sub in range(8):
            pi[t * 8 + sub] = sub * 16 + t
    return np.ascontiguousarray(h[pi, :])


_NC_CACHE = {}


def _build_program(variant=""):
    if variant in _NC_CACHE:
        return _NC_CACHE[variant]

    nc = bacc.Bacc("TRN2", target_bir_lowering=False, debug=False)

    d_pairs = nc.dram_tensor("pairs", [128, PAIRS_W], DT.int32, kind="ExternalInput")
    d_xT = nc.dram_tensor("xT", [128, KC * BATCH], DT.float16, kind="ExternalInput")
    d_suhT = nc.dram_tensor("suhT", [128, KC], DT.float16, kind="ExternalInput")
    d_H = nc.dram_tensor("Hmat", [128, 128], DT.float32, kind="ExternalInput")
    d_HP = nc.dram_tensor("HP", [128, 128], DT.float32, kind="ExternalInput")
    d_ident = nc.dram_tensor("ident8", [8, 8], DT.float32, kind="ExternalInput")
    d_svh = nc.dram_tensor("svhb", [8, NC_COLS], DT.float16, kind="ExternalInput")
    d_bias = nc.dram_tensor("biasb", [8, NC_COLS], DT.float16, kind="ExternalInput")
    d_out = nc.dram_tensor("out", [8, NC_COLS], DT.float16, kind="ExternalOutput")

    with tile.TileContext(nc) as tc:
        with (
            tc.tile_pool(name="const", bufs=1) as cpool,
            tc.tile_pool(name="pairs", bufs=3) as ppool,
            tc.tile_pool(name="stp", bufs=2) as stpool,
            tc.tile_pool(name="st2p", bufs=2) as st2pool,
            tc.tile_pool(name="h32p", bufs=2) as h32pool,
            tc.tile_pool(name="tail", bufs=2) as tailpool,
            tc.tile_pool(name="zslab", bufs=2) as zpool,
            tc.tile_pool(name="zslab1", bufs=1) as zpool1,
            tc.tile_pool(name="outp", bufs=1) as opool,
            tc.tile_pool(name="psum", bufs=2, space="PSUM") as pspool,
            tc.tile_pool(name="psum_s", bufs=2, space="PSUM") as pspool_s,
        ):
            # ---- slab-0 pairs DMA first: the decode pipeline's critical path ----
            t_pairs0 = ppool.tile([128, 3 * KC * 32], DT.int32, tag="pairs", name="t_pairs0")
            w0 = 3 * KC * SLABS[0][1]
            wc0 = w0 // 3
            for cpl in range(3):
                nc.sync.dma_start(
                    t_pairs0[:, cpl * wc0 : (cpl + 1) * wc0],
                    d_pairs[:, SLAB_OFF[0] + cpl * wc0 : SLAB_OFF[0] + (cpl + 1) * wc0],
                )

            # ---- constants / small inputs ----
            t_xT = cpool.tile([128, KC * BATCH], DT.float16, tag="xT")
            t_suhT = cpool.tile([128, KC], DT.float16, tag="suhT")
            t_H = cpool.tile([128, 128], DT.float32, tag="H")
            t_HP = cpool.tile([128, 128], DT.float32, tag="HP")
            t_id8 = cpool.tile([8, 8], DT.float32, tag="id8")
            t_svh = cpool.tile([8, NC_COLS], DT.float16, tag="svh")
            t_bias = cpool.tile([8, NC_COLS], DT.float16, tag="bias")
            nc.sync.dma_start(t_xT[:], d_xT[:])
            nc.sync.dma_start(t_suhT[:], d_suhT[:])
            nc.sync.dma_start(t_H[:], d_H[:])
            nc.sync.dma_start(t_HP[:], d_HP[:])
            nc.sync.dma_start(t_id8[:], d_ident[:])
            nc.sync.dma_start(t_svh[:], d_svh[:])
            nc.sync.dma_start(t_bias[:], d_bias[:])

            t_q = cpool.tile([128, 1], DT.int32, tag="cq")
            nc.vector.memset(t_q[:], LCG_Q)
            t_delta = cpool.tile([128, 1], DT.float32, tag="cdelta")
            nc.vector.memset(t_delta[:], float(DELTA16))
            t_rho = cpool.tile([128, 1], DT.float32, tag="crho")
            nc.vector.memset(t_rho[:], float(RHO16))

            # ---- input rotation: xhT[j, kc*8+b] ----
            t_xsT = cpool.tile([128, KC * BATCH], DT.float32, tag="xsT")
            nc.vector.tensor_tensor(
                t_xsT[:].rearrange("p (kc b) -> p kc b", kc=KC),
                t_xT[:].rearrange("p (kc b) -> p kc b", kc=KC),
                t_suhT[:].unsqueeze(2).broadcast_to([128, KC, BATCH]),
                AL.mult,
            )
            ps_xh = pspool.tile([128, KC * BATCH], DT.float32, tag="ps_xh")
            nc.tensor.matmul(ps_xh[:], t_H[:], t_xsT[:], start=True, stop=True)
            t_xhT = cpool.tile([128, KC * BATCH], DT.float16, tag="xhT")
            nc.scalar.copy(t_xhT[:], ps_xh[:])

            t_out = opool.tile([8, NC_COLS], DT.float16, tag="outsb")
            t_yh = opool.tile([8, NC_COLS], DT.float16, tag="yhsb")

            # ---- software-pipelined decode over flat (slab, pair) items ----
            # Each pipeline item covers G=2 adjacent classes (same z half),
            # halving per-instruction overhead on ACT/Pool/DVE SEQs.
            G = 2
            items = []
            for s in range(len(SLABS)):
                for t0 in range(0, 16, G):
                    items.append((s, t0))
            NI = len(items)
            PER_SLAB = 16 // G

            slab_state = {}  # s -> dict(pairs tile, z tiles, psum)
            st_of = {}  # item idx -> tiles

            def ensure_slab(s):
                if s in slab_state:
                    return slab_state[s]
                tn0, tnw = SLABS[s]
                w = 3 * KC * tnw
                if s == 0:
                    ss = {"pairs": t_pairs0, "w": w}
                    slab_state[s] = ss
                    return ss
                tp = ppool.tile([128, 3 * KC * 32], DT.int32, tag="pairs", name="tpairs")
                wc = w // 3
                for cpl in range(3):
                    nc.sync.dma_start(
                        tp[:, cpl * wc : (cpl + 1) * wc],
                        d_pairs[:, SLAB_OFF[s] + cpl * wc : SLAB_OFF[s] + (cpl + 1) * wc],
                    )
                ss = {"pairs": tp, "w": w}
                slab_state[s] = ss
                return ss

            def emit_E(j):
                s, t0 = items[j]
                ss = ensure_slab(s)
                if t0 == 0 and s + 1 < len(SLABS):
                    ensure_slab(s + 1)  # prefetch next slab's pairs
                tn0, tnw = SLABS[s]
                fw = KC * tnw
                pv = ss["pairs"][:, : ss["w"]].rearrange(
                    "p (c kc tn) -> p c kc tn", c=3, kc=KC
                )
                t_st = stpool.tile([128, G * KC * 32], DT.int32, tag="st", name="t_st")
                for g in range(G):
                    c, r = CLS[t0 + g]
                    nc.vector.tensor_scalar(
                        t_st[:, g * fw : (g + 1) * fw], pv[:, c], 16 - r, 0xFFFF,
                        AL.logical_shift_right, AL.bitwise_and,
                    )
                st_of[j] = {"st": t_st, "fw": fw}

            def emit_D(j):
                io = st_of[j]
                gfw = G * io["fw"]
                t_st2 = st2pool.tile([128, G * KC * 32], DT.int32, tag="st2", name="t_st2")
                nc.scalar.activation(
                    t_st2[:, :gfw], io["st"][:, :gfw],
                    mybir.ActivationFunctionType.Identity,
                    bias=t_delta[:], scale=1.0,
                )
                io["st2"] = t_st2

            def emit_M(j):
                s, t0 = items[j]
                io = st_of[j]
                gfw = G * io["fw"]
                ss = slab_state[s]
                if "tza" not in ss:
                    ss["tza"] = zpool.tile([128, 8 * KC * 32], DT.int32, tag="za", name="tza")
                    ss["tzb"] = zpool1.tile([128, 8 * KC * 32], DT.int32, tag="zb", name="tzb")
                nc.gpsimd.tensor_tensor(
                    z_view(j), io["st2"][:, :gfw],
                    t_q[:].broadcast_to([128, gfw]), AL.mult,
                )

            def z_view(j):
                s, t0 = items[j]
                io = st_of[j]
                fw = io["fw"]
                tz = slab_state[s]["tza"] if t0 < 8 else slab_state[s]["tzb"]
                return tz[:, (t0 % 8) * fw : (t0 % 8 + G) * fw]

            def emit_K(j):
                zv = z_view(j)
                nc.vector.tensor_scalar(
                    zv, zv, int(MASK32), None, AL.bitwise_and
                )

            def emit_R(j):
                io = st_of[j]
                gfw = G * io["fw"]
                t_h32 = h32pool.tile([128, G * KC * 32], DT.int32, tag="h32", name="t_h32")
                zq_odd = z_view(j).bitcast(DT.int16).rearrange(
                    "p (n x) -> p x n", x=2
                )[:, 1]
                nc.scalar.activation(
                    t_h32[:, :gfw], zq_odd,
                    mybir.ActivationFunctionType.Identity,
                    bias=t_rho[:], scale=1.0,
                )
                io["h32"] = t_h32

            def emit_O(j):
                io = st_of[j]
                gfw = G * io["fw"]
                tz_odd = z_view(j).bitcast(DT.int16).rearrange(
                    "p (n x) -> p x n", x=2
                )[:, 1]
                h32_lo = io["h32"][:, :gfw].bitcast(DT.int16).rearrange(
                    "p (n x) -> p x n", x=2
                )[:, 0]
                nc.vector.tensor_scalar(
                    tz_odd, h32_lo, 0x8FFF, None, AL.bitwise_and
                )

            def emit_mms_pair(j):
                s, t0 = items[j]
                tn0, tnw = SLABS[s]
                fw = KC * tnw
                nb = tnw // 8
                ss = slab_state[s]
                if "ps_y" not in ss:
                    ss["ps_y"] = pspool.tile([8, 512], DT.float32, tag="ps_y", name="ps_y")
                ps_yv = ss["ps_y"][:, : tnw * 16]
                pv = ps_yv.rearrange("p (b t sub) -> p b t sub", b=nb, t=16, sub=8)
                tz = ss["tza"] if t0 < 8 else ss["tzb"]
                th = t0 % 8
                zf = tz[:, th * fw : (th + G) * fw].bitcast(DT.float16).rearrange(
                    "p (t kc b sub x) -> p kc x t b sub",
                    t=G, kc=KC, b=nb, sub=8, x=2,
                )
                outv = pv[:, :, t0 : t0 + G, :].rearrange("p b t sub -> p t b sub")
                n_mm = 2 * KC
                i_mm = 0
                for xi in range(2):
                    for kc in range(KC):
                        nc.tensor.matmul(
                            outv,
                            t_xhT[:, kc * BATCH : (kc + 1) * BATCH],
                            zf[:, kc, xi],
                            start=(i_mm == 0),
                            stop=(i_mm == n_mm - 1),
                            skip_group_check=True,
                        )
                        i_mm += 1

            def emit_tail(s):
                tn0, tnw = SLABS[s]
                nb = tnw // 8
                ss = slab_state[s]
                ps_yv = ss["ps_y"][:, : tnw * 16]
                t_y = tailpool.tile([8, 512], DT.float32, tag="ysb", name="t_y")
                nc.scalar.copy(t_y[:, : tnw * 16], ps_yv)
                for bb in range(nb):
                    nblk = (tn0 // 8) + bb
                    ps_t = pspool_s.tile([128, 8], DT.float32, tag="ps_t", name="ps_t")
                    nc.tensor.transpose(
                        ps_t[:], t_y[:, bb * 128 : (bb + 1) * 128], t_id8[:]
                    )
                    t_yT = tailpool.tile([128, 8], DT.float32, tag="yT", name="t_yT")
                    nc.vector.tensor_copy(t_yT[:], ps_t[:])
                    ps_h = pspool_s.tile([8, 128], DT.float32, tag="ps_h", name="ps_h")
                    nc.tensor.matmul(ps_h[:], t_yT[:], t_HP[:], start=True, stop=True)
                    nc.vector.tensor_copy(
                        t_yh[:, nblk * 128 : (nblk + 1) * 128], ps_h[:]
                    )
                c0 = (tn0 // 8) * 128
                c1 = c0 + nb * 128
                nc.vector.tensor_tensor(
                    t_yh[:, c0:c1], t_yh[:, c0:c1], t_svh[:, c0:c1], AL.mult
                )
                nc.vector.tensor_tensor(
                    t_out[:, c0:c1], t_yh[:, c0:c1], t_bias[:, c0:c1], AL.add
                )
                nc.sync.dma_start(d_out[:, c0:c1], t_out[:, c0:c1])

            ensure_slab(0)
            for i in range(NI + MAXLAG):
                if i < NI:
                    emit_E(i)
                if 0 <= i - LAG_D < NI:
                    emit_D(i - LAG_D)
                if 0 <= i - LAG_M < NI:
                    emit_M(i - LAG_M)
                if 0 <= i - LAG_K < NI:
                    emit_K(i - LAG_K)
                if 0 <= i - LAG_R < NI:
                    emit_R(i - LAG_R)
                if 0 <= i - LAG_O < NI:
                    j = i - LAG_O
                    emit_O(j)
                    emit_mms_pair(j)
                    s, t0 = items[j]
                    if t0 + G == 16 and s == len(SLABS) - 1:
                        emit_tail(s)
                    if t0 // G == TAIL_DELAY and s > 0:
                        emit_tail(s - 1)
                    del st_of[j]


    nc.compile()
    _NC_CACHE[variant] = nc
    return nc


def _prep_core_inputs(x, trellis, suh, svh, bias, core):
    tshard = trellis[:, core * TNC : (core + 1) * TNC, :]  # [256, 112, 48]
    j = np.arange(16)
    pairs = np.empty((128, PAIRS_W), dtype=np.int32)
    for s, (tn0, tnw) in enumerate(SLABS):
        for c in range(3):
            wA = (3 * j + c) % 48
            wB = (3 * j + c + 1) % 48
            plA = tshard[:, tn0 : tn0 + tnw, wA].astype(np.uint32)  # [256, tnw, 16]
            plB = tshard[:, tn0 : tn0 + tnw, wB].astype(np.uint32)
            pl = (plA << 16) | plB
            arr = pl.reshape(KC, 8, tnw, 16)  # [kc, tk8, tn, j]
            arr = arr.transpose(1, 3, 0, 2).reshape(128, KC * tnw)
            off = SLAB_OFF[s] + c * KC * tnw
            pairs[:, off : off + KC * tnw] = arr.view(np.int32)

    # xT[p, kc*8+b] = x[b, kc*128+p]
    xT = np.ascontiguousarray(
        x.reshape(BATCH, KC, 128).transpose(2, 1, 0).reshape(128, KC * BATCH)
    )
    suhT = np.ascontiguousarray(suh.reshape(KC, 128).T)  # [128, 32]

    svh_s = svh[core * NC_COLS : (core + 1) * NC_COLS].astype(np.float16)
    bias_s = bias[core * NC_COLS : (core + 1) * NC_COLS].astype(np.float16)

    return {
        "pairs": pairs,
        "xT": xT,
        "suhT": suhT,
        "Hmat": _hadamard128(),
        "HP": _perm_h(),
        "ident8": np.eye(8, dtype=np.float32),
        "svhb": np.ascontiguousarray(np.broadcast_to(svh_s, (8, NC_COLS))),
        "biasb": np.ascontiguousarray(np.broadcast_to(bias_s, (8, NC_COLS))),
    }


def kernel(x, trellis, suh, svh, bias):
    x = np.asarray(x)
    trellis = np.asarray(trellis).astype(np.uint16)
    suh = np.asarray(suh)
    svh = np.asarray(svh)
    bias = np.asarray(bias)

    nc = _build_program()
    in_maps = [
        _prep_core_inputs(x, trellis, suh, svh, bias, core) for core in range(NCORES)
    ]
    res = run_bass_kernel_spmd(nc, in_maps, core_ids=list(range(NCORES)))
    global LAST_RUN
    LAST_RUN = res
    out = np.concatenate([res.results[c]["out"] for c in range(NCORES)], axis=1)
    return out.astype(np.float16)


LAST_RUN = None


if __name__ == "__main__":
    import reference as ref
    import jax.numpy as jnp

    inputs = {k: np.asarray(v) for k, v in ref.setup_inputs().items()}
    expected = np.asarray(ref.reference(**{k: jnp.asarray(v) for k, v in inputs.items()}))
    got = kernel(**inputs)
    e = np.linalg.norm(got.astype(np.float32) - expected.astype(np.float32))
    n = np.linalg.norm(expected.astype(np.float32))
    print("Relative error:", e / n)


# revision 18
# speedup vs baseline: 1.4289x; 1.0059x over previous
"""EXL3 trellis-quantized linear layer on 8 Trainium2 NeuronCores.

y = Had(Had(x*suh) @ dequant(trellis)) * svh + bias

Sharding: column-parallel over output features (N). Each of the 8 cores
dequants and multiplies its 1792-column shard; host concatenates.

Decode pipeline per weight (t = column-within-tile class, fixed shift r):
    pair  = (A << 16) | B                       A,B = trellis word pair
    state = (pair >> (16-r)) & 0xFFFF           DVE (i32 bitwise shr+and)
    st2   = state + DELTA16                     ACT (exact, < 2^17)
    g1    = st2 * Q   mod 2^32                  Pool TT (int32 exact)
    z     = g1 & 0x8FFF8FFF                     DVE
    z.hi  = (g1.hi + RHO16) & 0x8FFF            ACT add + DVE odd-mask
where DELTA16*Q == D (mod 2^16) and RHO16 corrects the high half, so
z == state*Q + D (mod 2^32) masked, exactly.  The masked z tile is bitcast
to fp16 and streamed to the PE as two rhs streams (lo/hi interleaved)
accumulating into the same PSUM bank.

The 64 (slab, class) items are software-pipelined: each engine's in-order
queue sees stage k of item i next to stage k-1 of item i+1, so no engine
stalls waiting for a cross-engine dependency of the same item.

Weight (j,t) of tile (Tk,Tn) sits at W[16Tk+j, 16Tn+t], so an output
column's weights share one t class. PSUM columns are produced t-major and
the output Hadamard uses a row-permuted H to compensate.
"""

import sys

if "/opt/trn_rl_repo" not in sys.path:
    sys.path.insert(0, "/opt/trn_rl_repo")

import numpy as np

import concourse.bacc as bacc
import concourse.mybir as mybir
from concourse import tile
from concourse.bass_utils import run_bass_kernel_spmd

AL = mybir.AluOpType
DT = mybir.dt

# problem geometry (hardcoded per contest contract)
K = 4096
N = 14336
BATCH = 8
NCORES = 8
TNC = (N // 16) // NCORES  # 112 trellis tile-cols per core
NC_COLS = TNC * 16  # 1792 out features per core
SLABS = [(0, 16), (16, 32), (48, 32), (80, 32)]  # (Tn offset, width)
KC = 32  # 128-row k-chunks

LCG_Q = 89226354
LCG_D = 64248484
DELTA16 = 14306  # DELTA16*Q ≡ D (mod 2^16)
RHO16 = 53288  # ((D - DELTA16*Q) >> 16) mod 2^16
MASK32 = np.int32(np.uint32(0x8FFF8FFF).astype(np.int64) - (1 << 32))

# stage lags for the software pipeline (item i emits stage S at i+LAG[S])
LAG_D = 1  # ACT delta
LAG_M = 2  # Pool mult
LAG_R = 3  # ACT rho (reads unmasked odd halves: must precede K)
LAG_K = 4  # DVE mask (in place on z)
LAG_O = 5  # DVE odd write-back
MAXLAG = 5
TAIL_DELAY = 5  # pair-slots into the next slab before emitting a slab's tail

# per-class constants
CLS = []
for t in range(16):
    c = (3 * t) // 16
    r = 3 * t - 16 * c
    CLS.append((c, r))

# slab-major pairs layout offsets (in i32 elements per partition)
SLAB_OFF = []
_off = 0
for _tn0, _tnw in SLABS:
    SLAB_OFF.append(_off)
    _off += 3 * KC * _tnw
PAIRS_W = _off  # 3*KC*TNC


def _hadamard128():
    h = np.array([[1.0]], dtype=np.float64)
    while h.shape[0] < 128:
        h = np.block([[h, h], [h, -h]])
    return (h / np.sqrt(128.0)).astype(np.float32)


def _perm_h():
    # psum col f' = t*8 + sub  <->  true in-block col sub*16 + t
    h = _hadamard128()
    pi = np.zeros(128, dtype=np.int64)
    for t in range(16):
        for sub in range(8):
            pi[t * 8 + sub] = sub * 16 + t
    return np.ascontiguousarray(h[pi, :])


_NC_CACHE = {}


def _build_program(variant=""):
    if variant in _NC_CACHE:
        return _NC_CACHE[variant]

    nc = bacc.Bacc("TRN2", target_bir_lowering=False, debug=False)

    d_pairs = nc.dram_tensor("pairs", [128, PAIRS_W], DT.int32, kind="ExternalInput")
    d_xT = nc.dram_tensor("xT", [128, KC * BATCH], DT.float16, kind="ExternalInput")
    d_suhT = nc.dram_tensor("suhT", [128, KC], DT.float16, kind="ExternalInput")
    d_H = nc.dram_tensor("Hmat", [128, 128], DT.float32, kind="ExternalInput")
    d_HPS = nc.dram_tensor("HPS", [128, (NC_COLS // 128) * 128], DT.float32, kind="ExternalInput")
    d_ident = nc.dram_tensor("ident8", [8, 8], DT.float32, kind="ExternalInput")
    d_bias = nc.dram_tensor("biasb", [8, NC_COLS], DT.float16, kind="ExternalInput")
    d_out = nc.dram_tensor("out", [8, NC_COLS], DT.float16, kind="ExternalOutput")

    with tile.TileContext(nc) as tc:
        with (
            tc.tile_pool(name="const", bufs=1) as cpool,
            tc.tile_pool(name="pairs", bufs=3) as ppool,
            tc.tile_pool(name="stp", bufs=2) as stpool,
            tc.tile_pool(name="st2p", bufs=2) as st2pool,
            tc.tile_pool(name="h32p", bufs=2) as h32pool,
            tc.tile_pool(name="tail", bufs=2) as tailpool,
            tc.tile_pool(name="zslab", bufs=2) as zpool,
            tc.tile_pool(name="zslab1", bufs=1) as zpool1,
            tc.tile_pool(name="outp", bufs=1) as opool,
            tc.tile_pool(name="psum", bufs=2, space="PSUM") as pspool,
            tc.tile_pool(name="psum_s", bufs=2, space="PSUM") as pspool_s,
        ):
            # ---- slab-0 pairs DMA first: the decode pipeline's critical path ----
            t_pairs0 = ppool.tile([128, 3 * KC * 32], DT.int32, tag="pairs", name="t_pairs0")
            w0 = 3 * KC * SLABS[0][1]
            wc0 = w0 // 3
            for cpl in range(3):
                nc.sync.dma_start(
                    t_pairs0[:, cpl * wc0 : (cpl + 1) * wc0],
                    d_pairs[:, SLAB_OFF[0] + cpl * wc0 : SLAB_OFF[0] + (cpl + 1) * wc0],
                )

            # ---- constants / small inputs ----
            t_xT = cpool.tile([128, KC * BATCH], DT.float16, tag="xT")
            t_suhT = cpool.tile([128, KC], DT.float16, tag="suhT")
            t_H = cpool.tile([128, 128], DT.float32, tag="H")
            t_HPS = cpool.tile([128, (NC_COLS // 128) * 128], DT.float32, tag="HPS")
            t_id8 = cpool.tile([8, 8], DT.float32, tag="id8")
            t_bias = cpool.tile([8, NC_COLS], DT.float16, tag="bias")
            nc.sync.dma_start(t_xT[:], d_xT[:])
            nc.sync.dma_start(t_suhT[:], d_suhT[:])
            nc.sync.dma_start(t_H[:], d_H[:])
            nc.sync.dma_start(t_HPS[:], d_HPS[:])
            nc.sync.dma_start(t_id8[:], d_ident[:])
            nc.sync.dma_start(t_bias[:], d_bias[:])

            t_q = cpool.tile([128, 1], DT.int32, tag="cq")
            nc.vector.memset(t_q[:], LCG_Q)
            t_delta = cpool.tile([128, 1], DT.float32, tag="cdelta")
            nc.vector.memset(t_delta[:], float(DELTA16))
            t_rho = cpool.tile([128, 1], DT.float32, tag="crho")
            nc.vector.memset(t_rho[:], float(RHO16))

            # ---- input rotation: xhT[j, kc*8+b] ----
            t_xsT = cpool.tile([128, KC * BATCH], DT.float32, tag="xsT")
            nc.vector.tensor_tensor(
                t_xsT[:].rearrange("p (kc b) -> p kc b", kc=KC),
                t_xT[:].rearrange("p (kc b) -> p kc b", kc=KC),
                t_suhT[:].unsqueeze(2).broadcast_to([128, KC, BATCH]),
                AL.mult,
            )
            ps_xh = pspool.tile([128, KC * BATCH], DT.float32, tag="ps_xh")
            nc.tensor.matmul(ps_xh[:], t_H[:], t_xsT[:], start=True, stop=True)
            t_xhT = cpool.tile([128, KC * BATCH], DT.float16, tag="xhT")
            nc.scalar.copy(t_xhT[:], ps_xh[:])

            t_out = opool.tile([8, NC_COLS], DT.float16, tag="outsb")

            # ---- software-pipelined decode over flat (slab, pair) items ----
            # Each pipeline item covers G=2 adjacent classes (same z half),
            # halving per-instruction overhead on ACT/Pool/DVE SEQs.
            G = 2
            items = []
            for s in range(len(SLABS)):
                for t0 in range(0, 16, G):
                    items.append((s, t0))
            NI = len(items)
            PER_SLAB = 16 // G

            slab_state = {}  # s -> dict(pairs tile, z tiles, psum)
            st_of = {}  # item idx -> tiles

            def ensure_slab(s):
                if s in slab_state:
                    return slab_state[s]
                tn0, tnw = SLABS[s]
                w = 3 * KC * tnw
                if s == 0:
                    ss = {"pairs": t_pairs0, "w": w}
                    slab_state[s] = ss
                    return ss
                tp = ppool.tile([128, 3 * KC * 32], DT.int32, tag="pairs", name="tpairs")
                wc = w // 3
                for cpl in range(3):
                    nc.sync.dma_start(
                        tp[:, cpl * wc : (cpl + 1) * wc],
                        d_pairs[:, SLAB_OFF[s] + cpl * wc : SLAB_OFF[s] + (cpl + 1) * wc],
                    )
                ss = {"pairs": tp, "w": w}
                slab_state[s] = ss
                return ss

            def emit_E(j):
                s, t0 = items[j]
                ss = ensure_slab(s)
                if t0 == 0 and s + 1 < len(SLABS):
                    ensure_slab(s + 1)  # prefetch next slab's pairs
                tn0, tnw = SLABS[s]
                fw = KC * tnw
                pv = ss["pairs"][:, : ss["w"]].rearrange(
                    "p (c kc tn) -> p c kc tn", c=3, kc=KC
                )
                t_st = stpool.tile([128, G * KC * 32], DT.int32, tag="st", name="t_st")
                for g in range(G):
                    c, r = CLS[t0 + g]
                    nc.vector.tensor_scalar(
                        t_st[:, g * fw : (g + 1) * fw], pv[:, c], 16 - r, 0xFFFF,
                        AL.logical_shift_right, AL.bitwise_and,
                    )
                st_of[j] = {"st": t_st, "fw": fw}

            def emit_D(j):
                io = st_of[j]
                gfw = G * io["fw"]
                t_st2 = st2pool.tile([128, G * KC * 32], DT.int32, tag="st2", name="t_st2")
                nc.scalar.activation(
                    t_st2[:, :gfw], io["st"][:, :gfw],
                    mybir.ActivationFunctionType.Identity,
                    bias=t_delta[:], scale=1.0,
                )
                io["st2"] = t_st2

            def emit_M(j):
                s, t0 = items[j]
                io = st_of[j]
                gfw = G * io["fw"]
                ss = slab_state[s]
                if "tza" not in ss:
                    ss["tza"] = zpool.tile([128, 8 * KC * 32], DT.int32, tag="za", name="tza")
                    ss["tzb"] = zpool1.tile([128, 8 * KC * 32], DT.int32, tag="zb", name="tzb")
                nc.gpsimd.tensor_tensor(
                    z_view(j), io["st2"][:, :gfw],
                    t_q[:].broadcast_to([128, gfw]), AL.mult,
                )

            def z_view(j):
                s, t0 = items[j]
                io = st_of[j]
                fw = io["fw"]
                tz = slab_state[s]["tza"] if t0 < 8 else slab_state[s]["tzb"]
                return tz[:, (t0 % 8) * fw : (t0 % 8 + G) * fw]

            def emit_K(j):
                zv = z_view(j)
                nc.vector.tensor_scalar(
                    zv, zv, int(MASK32), None, AL.bitwise_and
                )

            def emit_R(j):
                io = st_of[j]
                gfw = G * io["fw"]
                t_h32 = h32pool.tile([128, G * KC * 32], DT.int32, tag="h32", name="t_h32")
                zq_odd = z_view(j).bitcast(DT.int16).rearrange(
                    "p (n x) -> p x n", x=2
                )[:, 1]
                nc.scalar.activation(
                    t_h32[:, :gfw], zq_odd,
                    mybir.ActivationFunctionType.Identity,
                    bias=t_rho[:], scale=1.0,
                )
                io["h32"] = t_h32

            def emit_O(j):
                io = st_of[j]
                gfw = G * io["fw"]
                tz_odd = z_view(j).bitcast(DT.int16).rearrange(
                    "p (n x) -> p x n", x=2
                )[:, 1]
                h32_lo = io["h32"][:, :gfw].bitcast(DT.int16).rearrange(
                    "p (n x) -> p x n", x=2
                )[:, 0]
                nc.vector.tensor_scalar(
                    tz_odd, h32_lo, 0x8FFF, None, AL.bitwise_and
                )

            def emit_mms_pair(j):
                s, t0 = items[j]
                tn0, tnw = SLABS[s]
                fw = KC * tnw
                nb = tnw // 8
                ss = slab_state[s]
                if "ps_y" not in ss:
                    ss["ps_y"] = pspool.tile([8, 512], DT.float32, tag="ps_y", name="ps_y")
                ps_yv = ss["ps_y"][:, : tnw * 16]
                pv = ps_yv.rearrange("p (b t sub) -> p b t sub", b=nb, t=16, sub=8)
                tz = ss["tza"] if t0 < 8 else ss["tzb"]
                th = t0 % 8
                zf = tz[:, th * fw : (th + G) * fw].bitcast(DT.float16).rearrange(
                    "p (t kc b sub x) -> p kc x t b sub",
                    t=G, kc=KC, b=nb, sub=8, x=2,
                )
                outv = pv[:, :, t0 : t0 + G, :].rearrange("p b t sub -> p t b sub")
                n_mm = 2 * KC
                i_mm = 0
                for xi in range(2):
                    for kc in range(KC):
                        nc.tensor.matmul(
                            outv,
                            t_xhT[:, kc * BATCH : (kc + 1) * BATCH],
                            zf[:, kc, xi],
                            start=(i_mm == 0),
                            stop=(i_mm == n_mm - 1),
                            skip_group_check=True,
                        )
                        i_mm += 1

            def emit_tail(s):
                tn0, tnw = SLABS[s]
                nb = tnw // 8
                last = s == len(SLABS) - 1
                ss = slab_state[s]
                ps_yv = ss["ps_y"][:, : tnw * 16]
                t_y = tailpool.tile([8, 512], DT.float32, tag="ysb", name="t_y")
                nc.scalar.copy(t_y[:, : tnw * 16], ps_yv)
                for bb in range(nb):
                    nblk = (tn0 // 8) + bb
                    ps_t = pspool_s.tile([128, 8], DT.float32, tag="ps_t", name="ps_t")
                    nc.tensor.transpose(
                        ps_t[:], t_y[:, bb * 128 : (bb + 1) * 128], t_id8[:]
                    )
                    t_yT = tailpool.tile([128, 8], DT.float32, tag="yT", name="t_yT")
                    nc.vector.tensor_copy(t_yT[:], ps_t[:])
                    ps_h = pspool_s.tile([8, 128], DT.float32, tag="ps_h", name="ps_h")
                    nc.tensor.matmul(
                        ps_h[:], t_yT[:], t_HPS[:, nblk * 128 : (nblk + 1) * 128],
                        start=True, stop=True, skip_group_check=True,
                    )
                    nc.vector.tensor_tensor(
                        t_out[:, nblk * 128 : (nblk + 1) * 128], ps_h[:],
                        t_bias[:, nblk * 128 : (nblk + 1) * 128], AL.add,
                    )
                c0 = (tn0 // 8) * 128
                c1 = c0 + nb * 128
                nc.sync.dma_start(d_out[:, c0:c1], t_out[:, c0:c1])

            ensure_slab(0)
            for i in range(NI + MAXLAG):
                if i < NI:
                    emit_E(i)
                if 0 <= i - LAG_D < NI:
                    emit_D(i - LAG_D)
                if 0 <= i - LAG_M < NI:
                    emit_M(i - LAG_M)
                if 0 <= i - LAG_K < NI:
                    emit_K(i - LAG_K)
                if 0 <= i - LAG_R < NI:
                    emit_R(i - LAG_R)
                if 0 <= i - LAG_O < NI:
                    j = i - LAG_O
                    emit_O(j)
                    emit_mms_pair(j)
                    s, t0 = items[j]
                    if t0 + G == 16 and s == len(SLABS) - 1:
                        emit_tail(s)
                    if t0 // G == TAIL_DELAY and s > 0:
                        emit_tail(s - 1)
                    del st_of[j]


    nc.compile()
    _NC_CACHE[variant] = nc
    return nc


def _prep_core_inputs(x, trellis, suh, svh, bias, core):
    tshard = trellis[:, core * TNC : (core + 1) * TNC, :]  # [256, 112, 48]
    j = np.arange(16)
    pairs = np.empty((128, PAIRS_W), dtype=np.int32)
    for s, (tn0, tnw) in enumerate(SLABS):
        for c in range(3):
            wA = (3 * j + c) % 48
            wB = (3 * j + c + 1) % 48
            plA = tshard[:, tn0 : tn0 + tnw, wA].astype(np.uint32)  # [256, tnw, 16]
            plB = tshard[:, tn0 : tn0 + tnw, wB].astype(np.uint32)
            pl = (plA << 16) | plB
            arr = pl.reshape(KC, 8, tnw, 16)  # [kc, tk8, tn, j]
            arr = arr.transpose(1, 3, 0, 2).reshape(128, KC * tnw)
            off = SLAB_OFF[s] + c * KC * tnw
            pairs[:, off : off + KC * tnw] = arr.view(np.int32)

    # xT[p, kc*8+b] = x[b, kc*128+p]
    xT = np.ascontiguousarray(
        x.reshape(BATCH, KC, 128).transpose(2, 1, 0).reshape(128, KC * BATCH)
    )
    suhT = np.ascontiguousarray(suh.reshape(KC, 128).T)  # [128, 32]

    svh_s = svh[core * NC_COLS : (core + 1) * NC_COLS].astype(np.float32)
    bias_s = bias[core * NC_COLS : (core + 1) * NC_COLS].astype(np.float16)
    hp = _perm_h()
    hps = np.empty((128, NC_COLS), dtype=np.float32)
    for nblk in range(NC_COLS // 128):
        hps[:, nblk * 128 : (nblk + 1) * 128] = hp * svh_s[None, nblk * 128 : (nblk + 1) * 128]

    return {
        "pairs": pairs,
        "xT": xT,
        "suhT": suhT,
        "Hmat": _hadamard128(),
        "HPS": hps,
        "ident8": np.eye(8, dtype=np.float32),
        "biasb": np.ascontiguousarray(np.broadcast_to(bias_s, (8, NC_COLS))),
    }


def kernel(x, trellis, suh, svh, bias):
    x = np.asarray(x)
    trellis = np.asarray(trellis).astype(np.uint16)
    suh = np.asarray(suh)
    svh = np.asarray(svh)
    bias = np.asarray(bias)

    nc = _build_program()
    in_maps = [
        _prep_core_inputs(x, trellis, suh, svh, bias, core) for core in range(NCORES)
    ]
    res = run_bass_kernel_spmd(nc, in_maps, core_ids=list(range(NCORES)))
    global LAST_RUN
    LAST_RUN = res
    out = np.concatenate([res.results[c]["out"] for c in range(NCORES)], axis=1)
    return out.astype(np.float16)


LAST_RUN = None


if __name__ == "__main__":
    import reference as ref
    import jax.numpy as jnp

    inputs = {k: np.asarray(v) for k, v in ref.setup_inputs().items()}
    expected = np.asarray(ref.reference(**{k: jnp.asarray(v) for k, v in inputs.items()}))
    got = kernel(**inputs)
    e = np.linalg.norm(got.astype(np.float32) - expected.astype(np.float32))
    n = np.linalg.norm(expected.astype(np.float32))
    print("Relative error:", e / n)
